# revision 1
# baseline (speedup 1.0000x reference)
"""Trainium2 Bass kernel for nn_CrossAttention (B=8, C=256, H=W=64, inter=32).

Math (per batch sample b):
    Q = Wq @ xg + bq          (32, 4096)   xg = gaf_features[b]  (256, 4096)
    K = Wk @ xm + bk          (32, 4096)   xm = mtf_features[b]
    V = Wv @ xm + bv          (32, 4096)
    L[k, q]   = sum_c K[c, k] Q[c, q]          (4096, 4096)
    A[k, q]   = exp(L[k, q]) / sum_q' exp(L[k, q'])     (softmax over q)
    out[c, q] = sum_k V[c, k] A[k, q]          (32, 4096)
    res       = gamma * (Wo @ out + bo) + xg   (256, 4096)

Sharding: data-parallel over batch — core i handles sample i (8 cores, B=8).

Per-core kernel structure:
  - k-dim processed in 32 tiles of 128 (k on PSUM/SBUF partitions)
  - L computed per k-tile in PSUM chunks [1536, 1536, 1024] (double buffered,
    6 banks), drained by ScalarE exp with fused per-partition row-sum
    (accum_out) giving Z for free.  Softmax max-subtraction is skipped: the
    logits here are bounded (|L| < ~10) by construction, exp is exact fp32.
  - 1/Z folded into V^T rows (per-partition scalar mul), so P = exp(L) is
    consumed unmodified by the output matmul.
  - out accumulated over all 32 k-tiles directly in PSUM (2 banks) using
    column-tiled matmuls (tile_position=(0,32j)) packing the 4 q-chunk groups
    into the 128 partitions; a zero dummy matmul initializes the banks.
  - epilogue: row-tiled Wo projection (tile_position=(32j,0)) + residual.
"""

import numpy as np

C = 256          # in channels
D = 32           # inter channels
HW = 4096        # H*W
P = 128
NKT = HW // P    # 32 k-tiles
NQC = HW // 512  # 8 q-chunks of 512
B = 8
H = W = 64

# L-chunk split per k-tile: offsets/lengths in q, each a multiple of 512.
L_CHUNKS = [(0, 1536), (1536, 1536), (3072, 1024)]

_CACHE = {}
PROFILE = False           # set True (e.g. from test.py) to collect a trace
LAST_EXEC_NS = None
LAST_RESULTS = None


def _build_nc():
    import concourse.tile as tile
    from concourse import bacc, mybir

    F32 = mybir.dt.float32
    BF16 = mybir.dt.bfloat16
    Act = mybir.ActivationFunctionType
    Alu = mybir.AluOpType

    nc = bacc.Bacc()

    xg_h = nc.declare_dram_parameter("xg", [C, HW], F32, isOutput=False)
    xm_h = nc.declare_dram_parameter("xm", [C, HW], F32, isOutput=False)
    # wqkv: [C, 3D] = WqT | WkT | WvT packed (single DMA -> single producer)
    wqkv_h = nc.declare_dram_parameter("wqkv", [C, 3 * D], F32, isOutput=False)
    wo_h = nc.declare_dram_parameter("wo", [D, C], F32, isOutput=False)  # Wo^T
    # consts [P, 37]: bvb(0:32), bo2(32:34), gmb(34:35), bq(35) rows 0:32,
    # bk(36) rows 0:32
    cst_h = nc.declare_dram_parameter("cst", [P, 37], F32, isOutput=False)
    res_h = nc.declare_dram_parameter("res", [C, HW], F32, isOutput=True)

    xg = xg_h[:].rearrange("(o p) q -> p o q", p=P)   # c = o*128 + p
    xm = xm_h[:].rearrange("(o p) q -> p o q", p=P)
    res = res_h[:].rearrange("(o p) q -> p o q", p=P)
    wqkv = wqkv_h[:].rearrange("(o p) d -> p o d", p=P)

    with tile.TileContext(nc) as tc:
        with (
            tc.tile_pool(name="singles", bufs=1) as singles,
            tc.tile_pool(name="ppool", bufs=3) as ppool,
            tc.tile_pool(name="lpool", bufs=2, space="PSUM") as lpool,
            tc.tile_pool(name="opool", bufs=1, space="PSUM") as opool,
            tc.tile_pool(name="small", bufs=4) as small,
            tc.tile_pool(name="respool", bufs=4) as respool,
        ):
            # ---------------- constants ----------------
            wqkv_s = singles.tile([P, 2, 3 * D], F32, name="wqkv_s")
            nc.sync.dma_start(out=wqkv_s, in_=wqkv)
            wq_s = wqkv_s[:, :, 0:D]
            wk_s = wqkv_s[:, :, D : 2 * D]
            wv_s = wqkv_s[:, :, 2 * D : 3 * D]
            wo_s = singles.tile([P, C], F32, name="wo_s")
            for j in range(4):  # replicate Wo^T into the 4 partition strips
                nc.sync.dma_start(out=wo_s[32 * j : 32 * (j + 1), :], in_=wo_h[:])
            cst_s = singles.tile([P, 37], F32, name="cst_s")
            nc.sync.dma_start(out=cst_s, in_=cst_h[:])
            bvb_s = cst_s[:, 0:D]
            bo_s = cst_s[:, D : D + 2]
            gm_s = cst_s[:, D + 2 : D + 3]
            bq_s = cst_s[:D, D + 3 : D + 4]
            bk_s = cst_s[:D, D + 4 : D + 5]
            gbo_s = singles.tile([P, 2], F32, name="gbo_s")
            nc.vector.tensor_scalar_mul(gbo_s, bo_s, gm_s)  # gamma * bo
            zero_s = singles.tile([P, 512], F32, name="zero_s")
            nc.vector.memset(zero_s, 0.0)

            # input feature tiles (kept resident; xg also used for residual)
            xg_s = singles.tile([P, 2, HW], F32, name="xg_s")
            xm_s = singles.tile([P, 2, HW], F32, name="xm_s")

            # Q/K chunk tiles (separate tiles -> fine grained deps)
            q_tiles = [singles.tile([D, 512], BF16, name=f"q_t{i}") for i in range(NQC)]
            k_tiles = [singles.tile([D, 512], BF16, name=f"k_t{i}") for i in range(NQC)]
            vt_tiles = [
                singles.tile([P, D], F32, name=f"vt_t{t}") for t in range(NKT)
            ]

            # persistent col-packed output accumulator: strip j of bank b holds
            # out[:, 512*(4b+j) : 512*(4b+j)+512]
            out_ps = opool.tile([P, 1024], F32, name="out_ps")
            # dummy zero matmuls: clear has_written for both banks, data = 0
            for b in range(2):
                nc.tensor.matmul(
                    out=out_ps[:, 512 * b : 512 * (b + 1)],
                    lhsT=zero_s[:, :P],
                    rhs=zero_s[:, :512],
                    start=True,
                    stop=False,
                    skip_group_check=True,
                )

            # ---------------- load + projections (per 512-q chunk) ----------------
            for qc in range(NQC):
                sl = slice(512 * qc, 512 * (qc + 1))
                nc.sync.dma_start(out=xg_s[:, :, sl], in_=xg[:, :, sl])
                nc.sync.dma_start(out=xm_s[:, :, sl], in_=xm[:, :, sl])

                q_ps = lpool.tile([P, 512], F32, tag="lc", name="q_ps")
                nc.tensor.matmul(
                    out=q_ps[:D], lhsT=wq_s[:, 0, :], rhs=xg_s[:, 0, sl],
                    start=True, stop=False,
                )
                nc.tensor.matmul(
                    out=q_ps[:D], lhsT=wq_s[:, 1, :], rhs=xg_s[:, 1, sl],
                    start=False, stop=True,
                )
                nc.vector.tensor_scalar_add(q_tiles[qc], q_ps[:D], bq_s)

                k_ps = lpool.tile([P, 512], F32, tag="lc", name="k_ps")
                nc.tensor.matmul(
                    out=k_ps[:D], lhsT=wk_s[:, 0, :], rhs=xm_s[:, 0, sl],
                    start=True, stop=False,
                )
                nc.tensor.matmul(
                    out=k_ps[:D], lhsT=wk_s[:, 1, :], rhs=xm_s[:, 1, sl],
                    start=False, stop=True,
                )
                nc.vector.tensor_scalar_add(k_tiles[qc], k_ps[:D], bk_s)

                # V^T for the 4 k-tiles inside this chunk:
                # vt[k, c] = sum_ch xm[ch, k] * WvT[ch, c]  (+ bv broadcast)
                for t in range(4):
                    kt = 4 * qc + t
                    ksl = slice(P * kt, P * (kt + 1))
                    vt_ps = lpool.tile([P, D], F32, tag="lc", name="vt_ps")
                    nc.tensor.matmul(
                        out=vt_ps, lhsT=xm_s[:, 0, ksl], rhs=wv_s[:, 0, :],
                        start=True, stop=False,
                    )
                    nc.tensor.matmul(
                        out=vt_ps, lhsT=xm_s[:, 1, ksl], rhs=wv_s[:, 1, :],
                        start=False, stop=True,
                    )
                    nc.vector.tensor_add(vt_tiles[kt], vt_ps, bvb_s)

            # ---------------- main loop over k-tiles ----------------
            for kt in range(NKT):
                kq = kt // 4            # which K chunk tile
                ko = (kt % 4) * P       # offset inside it
                p_t = ppool.tile([P, HW], BF16, tag="p", name="p_t")
                zp = small.tile([P, len(L_CHUNKS)], F32, name="zp")
                for ci, (qoff, clen) in enumerate(L_CHUNKS):
                    l_ps = lpool.tile([P, 1536], F32, tag="lc", name="l_ps")
                    for j in range(clen // 512):
                        qi = (qoff + 512 * j) // 512
                        nc.tensor.matmul(
                            out=l_ps[:, 512 * j : 512 * (j + 1)],
                            lhsT=k_tiles[kq][:, ko : ko + P],
                            rhs=q_tiles[qi],
                            start=True,
                            stop=True,
                        )
                    nc.scalar.activation(
                        out=p_t[:, qoff : qoff + clen],
                        in_=l_ps[:, :clen],
                        func=Act.Exp,
                        accum_out=zp[:, ci : ci + 1],
                    )
                zs = small.tile([P, 1], F32, name="zs")
                nc.vector.reduce_sum(out=zs, in_=zp, axis=mybir.AxisListType.X)
                zr = small.tile([P, 1], F32, name="zr")
                nc.vector.reciprocal(out=zr, in_=zs)
                vts = small.tile([P, D], BF16, name="vts")
                nc.vector.tensor_scalar_mul(vts, vt_tiles[kt], zr)

                for b in range(2):
                    for j in range(4):
                        qi = 4 * b + j
                        nc.tensor.matmul(
                            out=out_ps[32 * j : 32 * (j + 1), 512 * b : 512 * (b + 1)],
                            lhsT=vts,
                            rhs=p_t[:, 512 * qi : 512 * (qi + 1)],
                            tile_position=(0, 32 * j),
                            start=False,
                            stop=(kt == NKT - 1 and j == 3),
                            skip_group_check=True,
                        )

            # ---------------- epilogue: Wo projection + residual ----------------
            out4_s = singles.tile([P, 1024], F32, name="out4_s")
            nc.vector.tensor_copy(out=out4_s, in_=out_ps)
            for h in range(2):          # co half
                for part in range(2):   # q-chunk within strip
                    for j in range(4):  # strip (row group)
                        qi = 4 * part + j
                        qsl = slice(512 * qi, 512 * (qi + 1))
                        o2_ps = lpool.tile([P, 512], F32, tag="lc", name="o2_ps")
                        nc.tensor.matmul(
                            out=o2_ps,
                            lhsT=wo_s[32 * j : 32 * (j + 1), P * h : P * (h + 1)],
                            rhs=out4_s[32 * j : 32 * (j + 1),
                                       512 * part : 512 * (part + 1)],
                            tile_position=(32 * j, 0),
                            start=True,
                            stop=True,
                        )
                        res_s = respool.tile([P, 512], F32, name="res_s")
                        # res = gamma*o2 + xg ; then += gamma*bo
                        nc.vector.scalar_tensor_tensor(
                            out=res_s,
                            in0=o2_ps,
                            scalar=gm_s,
                            op0=Alu.mult,
                            in1=xg_s[:, h, qsl],
                            op1=Alu.add,
                        )
                        nc.vector.tensor_scalar_add(
                            res_s, res_s, gbo_s[:, h : h + 1]
                        )
                        nc.sync.dma_start(out=res[:, h, qsl], in_=res_s)

    nc.finalize()
    return nc


def _get_nc():
    if "nc" not in _CACHE:
        _CACHE["nc"] = _build_nc()
    return _CACHE["nc"]


def _make_in_maps(gaf, mtf, Wq, bq, Wk, bk, Wv, bv, Wo, bo, gamma):
    f = np.float32
    wqkv = np.concatenate([Wq.T, Wk.T, Wv.T], axis=1).astype(f)   # (256, 96)
    wo = np.ascontiguousarray(Wo.T, dtype=f)                      # (32, 256)
    cst = np.zeros((P, 37), f)
    cst[:, 0:D] = np.broadcast_to(bv.reshape(1, D), (P, D))       # bvb
    cst[:, D:D + 2] = bo.reshape(2, P).T                          # bo2 [p, o]
    cst[:, D + 2] = np.asarray(gamma).reshape(-1)[0]              # gamma bcast
    cst[:D, D + 3] = bq                                           # bq
    cst[:D, D + 4] = bk                                           # bk
    shared = dict(wqkv=np.ascontiguousarray(wqkv), wo=wo, cst=cst)
    in_maps = []
    for b in range(B):
        m = dict(shared)
        m["xg"] = np.ascontiguousarray(gaf[b].reshape(C, HW), dtype=f)
        m["xm"] = np.ascontiguousarray(mtf[b].reshape(C, HW), dtype=f)
        in_maps.append(m)
    return in_maps


def kernel(gaf_features, mtf_features, Wq, bq, Wk, bk, Wv, bv, Wo, bo, gamma):
    global LAST_EXEC_NS, LAST_RESULTS
    from concourse.bass_utils import run_bass_kernel_spmd

    nc = _get_nc()
    in_maps = _make_in_maps(
        np.asarray(gaf_features), np.asarray(mtf_features),
        np.asarray(Wq), np.asarray(bq), np.asarray(Wk), np.asarray(bk),
        np.asarray(Wv), np.asarray(bv), np.asarray(Wo), np.asarray(bo),
        np.asarray(gamma),
    )
    core_ids = list(range(B))
    r = run_bass_kernel_spmd(nc, in_maps, core_ids, trace=PROFILE)
    LAST_EXEC_NS = r.exec_time_ns
    LAST_RESULTS = r
    out = np.stack([r.results[i]["res"] for i in range(B)], axis=0)
    return out.reshape(B, C, H, W).astype(np.float32)

